# revision 1
# baseline (speedup 1.0000x reference)
"""Multi-head attention (B=2, N=2048, EMB=1024, H=16, hd=64) on 8 TRN2 NeuronCores.

Sharding: tensor-parallel over heads. Each core owns 2 heads: it gets the
W_qkv columns (k|q|v sections) and W_out rows for those heads, computes
QKV projection + attention + its partial output projection, and the host
sums the 8 partials (the "all-reduce") and adds b_out.

Device kernel layout (per core), all matmuls bf16 with fp32 PSUM accumulation:
  - x is pre-transposed on host to xT [EMB, TOK] so the embedding dim lands on
    SBUF partitions (matmul contraction dim).
  - QKV^T is produced in [dims, tokens] layout: lhsT = W chunk, rhs = xT chunk.
    K/Q sections stay transposed ([hd, tok]) for the scores matmul; the V
    section is staged transposed then moved into [tok, hd] tiles (vaug) via
    row-packed plain matmuls against identity blocks (NOT transpose-mode
    matmuls: those run at the cold clock and cost ~600ns each).
  - scores^T chunks [k_tok 128, q 512] per head via row-packed (K=64) matmuls,
    exp on ScalarE straight out of PSUM (scale=1/8 folded in, no max
    subtraction needed: scores ~ N(0,1)), bf16 expT.
  - attn@V: lhsT = vaug [k_tok 128, 128-pad] (col 64 = ones -> row 64 of the
    output accumulates the softmax denominator), accumulated over 16 k chunks.
  - normalize (software-pipelined one unit behind the exp stream so ScalarE
    never idles at unit boundaries): psa evicted fp32; den rows DMA-packed to
    [2, 512]; 1/den = exp(-ln(den)) on ScalarE; partition-broadcast by a K=1
    ones-matmul; multiply on VectorE into A_norm [att 128, tok] bf16.
  - out projection: lhsT = A_norm chunk [128, 128], rhs = W_out shard
    [128, 512], PSUM -> SBUF bf16 -> DRAM partial [TOK, EMB] bf16.
"""

import os

import numpy as np
import ml_dtypes

B = 2
N = 2048
EMB = 1024
TOK = B * N  # 4096
HD = 64
H_PER_CORE = 2
DIMS = 3 * H_PER_CORE * HD  # 384 qkv cols per core
ATT_LOCAL = H_PER_CORE * HD  # 128
P = 128
EC = EMB // P  # 8 embedding chunks
TCQ = TOK // 512  # 8 token chunks for the qkv projection
KCH = N // P  # 16 key chunks per batch
QQ = N // 512  # 4 query quarters per batch
SCALE = HD ** -0.5

_CACHE = {}
LAST = {}


def _patch_act_tables():
    # Route Exp to natural_log_exp_and_others so the per-unit Ln calls and
    # the bulk Exp calls share one table set (no ACT_TABLE_LOAD thrash).
    # Entries keep their order, so act_func_set_id indices stay valid.
    import concourse.bacc as bacc_mod
    from concourse import mybir

    if getattr(bacc_mod, "_act_tables_patched", False):
        return
    orig = bacc_mod.get_activation_tables

    def patched(arch):
        t = orig(arch)
        E = mybir.ActivationFunctionType.Exp
        if "natural_log_exp_and_others" in t:
            for name, fns in t.items():
                if name != "natural_log_exp_and_others" and E in fns:
                    t[name] = fns - {E}
        return t

    bacc_mod.get_activation_tables = patched
    bacc_mod._act_tables_patched = True


def _build_graph():
    from concourse import bacc, mybir
    import concourse.tile as tile

    _patch_act_tables()

    nc = bacc.Bacc(
        "TRN2", target_bir_lowering=False, debug=False, num_devices=1
    )
    dt = mybir.dt
    xT = nc.dram_tensor("xT", [EMB, TOK], dt.bfloat16, kind="ExternalInput")
    wqkv = nc.dram_tensor("wqkv", [EMB, DIMS], dt.bfloat16, kind="ExternalInput")
    bqkv = nc.dram_tensor("bqkv", [DIMS], dt.float32, kind="ExternalInput")
    wout = nc.dram_tensor("wout", [ATT_LOCAL, EMB], dt.bfloat16, kind="ExternalInput")
    out = nc.dram_tensor("out", [TOK, EMB], dt.bfloat16, kind="ExternalOutput")

    with tile.TileContext(nc) as tc:
        _emit(tc, nc, xT, wqkv, bqkv, wout, out)
    nc.compile()
    return nc


def _emit(tc, nc, xT, wqkv, bqkv, wout, out):
    from contextlib import ExitStack
    import concourse.bass as bass
    from concourse import mybir
    from concourse.masks import make_identity

    dt = mybir.dt
    f32, bf16 = dt.float32, dt.bfloat16
    Exp = mybir.ActivationFunctionType.Exp
    Ln = mybir.ActivationFunctionType.Ln

    with ExitStack() as ctx:
        consts = ctx.enter_context(tc.tile_pool(name="consts", bufs=1))
        xt_pool = ctx.enter_context(tc.tile_pool(name="xt", bufs=3))
        persist = ctx.enter_context(tc.tile_pool(name="persist", bufs=1))
        expp = ctx.enter_context(tc.tile_pool(name="expp", bufs=12))
        small = ctx.enter_context(tc.tile_pool(name="small", bufs=10))
        outst = ctx.enter_context(tc.tile_pool(name="outst", bufs=4))
        ps_scores = ctx.enter_context(
            tc.tile_pool(name="ps_scores", bufs=2, space="PSUM")
        )
        ps_att = ctx.enter_context(tc.tile_pool(name="ps_att", bufs=2, space="PSUM"))
        ps_small = ctx.enter_context(
            tc.tile_pool(name="ps_small", bufs=2, space="PSUM")
        )

        # ---- constants / persistent tiles ----
        # warm up the exp table set as early as possible (one-time ~2.7us)
        warm = consts.tile([1, 8], f32, tag="warm")
        nc.vector.memset(warm, 1.0)
        nc.scalar.activation(out=warm, in_=warm, func=Ln, scale=1.0)
        nc.scalar.activation(out=warm, in_=warm, func=Exp, scale=1.0)

        w_sb = consts.tile([P, EC, DIMS], bf16, tag="w_sb")
        for e in range(EC):
            nc.sync.dma_start(out=w_sb[:, e, :], in_=wqkv[e * P : (e + 1) * P, :])
        bias_sb = consts.tile([P, 3], f32, tag="bias_sb")
        nc.sync.dma_start(out=bias_sb, in_=bqkv[:].rearrange("(c p) -> p c", p=P))
        wout_sb = consts.tile([P, EMB], bf16, tag="wout_sb")
        nc.sync.dma_start(out=wout_sb, in_=wout[:, :])
        ident = consts.tile([P, P], bf16, tag="ident")
        make_identity(nc, ident)
        ones64 = consts.tile([HD + 1, HD], bf16, tag="ones64")
        nc.vector.memset(ones64, 1.0)
        wm_ps = ps_small.tile([P, 512], f32, tag="ps_small", name="wm_ps")
        for _ in range(28):
            nc.tensor.matmul(
                wm_ps[:, 0:128], lhsT=ident, rhs=ident, start=True, stop=True
            )

        k_sb = persist.tile([P, TOK], bf16, tag="k_sb")
        q_sb = persist.tile([P, TOK], bf16, tag="q_sb")
        # [tok 128, b, chunk, h, hd+pad]: padded to 128 cols per head so the
        # attn@V weight load gets FWL
        vaug = persist.tile([P, B, KCH, H_PER_CORE, P], bf16, tag="vaug")
        anorm = persist.tile([P, TOK], bf16, tag="anorm")
        nc.vector.memset(vaug[:, :, :, :, :], 0.0)
        # ones column of V_aug (the softmax denominator accumulator row)
        nc.vector.memset(vaug[:, :, :, :, HD : HD + 1], 1.0)

        qkv_dst = (k_sb, q_sb)

        xts = {}

        def qkv_chunk(t):
            # K and Q sections for tokens t*512 .. (t+1)*512 (V is produced
            # separately, directly in [token, dim] orientation by v_chunk)
            xt = xt_pool.tile([P, EC, 512], bf16, tag="xt")
            for e in range(EC):
                nc.sync.dma_start(
                    out=xt[:, e, :], in_=xT[e * P : (e + 1) * P, bass.ts(t, 512)]
                )
            for d in range(2):
                ps = ps_small.tile([P, 512], f32, tag="ps_small")
                for e in range(EC):
                    nc.tensor.matmul(
                        ps,
                        lhsT=w_sb[:, e, d * P : (d + 1) * P],
                        rhs=xt[:, e, :],
                        start=(e == 0),
                        stop=(e == EC - 1),
                    )
                nc.vector.tensor_scalar_add(
                    out=qkv_dst[d][:, bass.ts(t, 512)],
                    in0=ps,
                    scalar1=bias_sb[:, d : d + 1],
                )
            xts[t] = xt

        def v_chunk(t, tci0=0, tci1=4):
            # V[tok, dim] for tokens t*512+tci*128 .. +128, straight into
            # vaug: lhsT = xT chunk [emb 128, tok 128] (FWL), rhs = Wv
            # section [emb 128, dims 128], accumulated over 8 emb chunks.
            for tci in range(tci0, tci1):
                gtok = t * 512 + tci * P
                b, i = gtok // N, (gtok % N) // P
                ps = ps_small.tile([P, 512], f32, tag="ps_small",
                                   name=f"v{t}_{tci}")
                for e in range(EC):
                    nc.tensor.matmul(
                        ps[:, 0:P],
                        lhsT=xts[t][:, e, tci * P : (tci + 1) * P],
                        rhs=w_sb[:, e, 2 * P : 3 * P],
                        start=(e == 0),
                        stop=(e == EC - 1),
                    )
                nc.vector.tensor_copy(
                    out=vaug[:, b, i, :, 0:HD],
                    in_=ps[:, 0:P].rearrange("p (h d) -> p h d", h=2),
                )

        def alloc_ps_a(b, qq):
            return [
                ps_att.tile([P, 512], f32, tag="ps_att", name=f"ps_a{b}_{qq}_{h}")
                for h in range(H_PER_CORE)
            ]

        def attention_chunks(b, qq, ps_a, i0, i1, filler=None):
            # attn@V for chunk i is emitted two chunks late: by the time the
            # in-order PE stream reaches it, exp(i) has long finished, so the
            # PE never sits on a semaphore between scores matmuls.
            # `filler` emits one slice of independent work per chunk, between
            # the scores and av groups, to keep other engines fed.
            qbase = b * N + qq * 512
            lag = []

            def attnv(i, ex):
                for h in range(H_PER_CORE):
                    nc.tensor.matmul(
                        ps_a[h],
                        lhsT=vaug[:, b, i, h, :],
                        rhs=ex[:, h * 512 : (h + 1) * 512],
                        start=(i == 0),
                        stop=(i == KCH - 1),
                    )

            for i in range(i0, i1):
                kbase = b * N + i * P
                ps_s = ps_scores.tile([P, 1024], f32, tag="ps_s")
                for h in range(H_PER_CORE):
                    nc.tensor.matmul(
                        ps_s[:, h * 512 : (h + 1) * 512],
                        lhsT=k_sb[h * HD : (h + 1) * HD, kbase : kbase + P],
                        rhs=q_sb[h * HD : (h + 1) * HD, qbase : qbase + 512],
                        start=True,
                        stop=True,
                    )
                ex = expp.tile([P, 1024], bf16, tag="expT")
                nc.scalar.activation(out=ex, in_=ps_s, func=Exp, scale=SCALE)
                lag.append((i, ex))
                if len(lag) > 2:
                    attnv(*lag.pop(0))
                if filler is not None:
                    filler(i - i0)
            for item in lag:
                attnv(*item)

        # per-unit finish state, produced by drain_unit / consumed by
        # finish_unit+outproj_unit one unit later
        fin = {}

        def drain_unit(b, qq, ps_a):
            # evict accumulators to SBUF (bf16) while PSUM banks free up
            psa_sb = []
            for h in range(H_PER_CORE):
                t = small.tile([HD + 1, 512], bf16, tag="psa_sb")
                nc.vector.tensor_copy(out=t, in_=ps_a[h][0 : HD + 1, :])
                psa_sb.append(t)
            fin[(b, qq)] = psa_sb

        def finish_unit(b, qq):
            # normalize: 1/den = exp(-ln(den)) on ScalarE (same table set as
            # the bulk exp); partition-broadcast via K=1 bf16 ones-matmul;
            # multiplies on VectorE.
            psa_sb = fin.pop((b, qq))
            qbase = b * N + qq * 512
            for h in range(H_PER_CORE):
                tln = small.tile([HD + 1, 512], f32, tag="tln")
                nc.scalar.activation(
                    out=tln[HD : HD + 1, :],
                    in_=psa_sb[h][HD : HD + 1, :],
                    func=Ln,
                )
                rcbf = small.tile([HD + 1, 512], bf16, tag="rcbf")
                nc.scalar.activation(
                    out=rcbf[HD : HD + 1, :],
                    in_=tln[HD : HD + 1, :],
                    func=Exp,
                    scale=-1.0,
                )
                rrep_ps = ps_small.tile(
                    [HD, 512], f32, tag="ps_small", name=f"rrep{b}_{qq}_{h}"
                )
                nc.tensor.matmul(
                    rrep_ps,
                    lhsT=ones64[HD : HD + 1, 0:HD],
                    rhs=rcbf[HD : HD + 1, :],
                    start=True,
                    stop=True,
                )
                if h == 0:
                    nc.vector.tensor_mul(
                        out=anorm[0:HD, qbase : qbase + 512],
                        in0=psa_sb[0][0:HD, :],
                        in1=rrep_ps,
                    )
                else:
                    # engine lanes cannot shift partitions; go through a
                    # partition-0 temp and DMA into partitions 64..127.
                    tmp = small.tile([HD, 512], bf16, tag="anorm_tmp")
                    nc.vector.tensor_mul(
                        out=tmp, in0=psa_sb[1][0:HD, :], in1=rrep_ps
                    )
                    nc.sync.dma_start(
                        out=anorm[HD : 2 * HD, qbase : qbase + 512], in_=tmp
                    )

        def outproj_unit(b, qq):
            qbase = b * N + qq * 512
            for tci in range(4):
                tok0 = qbase + tci * P
                ob = outst.tile([P, EMB], bf16, tag="outst")
                for e2 in range(2):
                    ps = ps_small.tile([P, 512], f32, tag="ps_small")
                    nc.tensor.matmul(
                        ps,
                        lhsT=anorm[:, tok0 : tok0 + P],
                        rhs=wout_sb[:, e2 * 512 : (e2 + 1) * 512],
                        start=True,
                        stop=True,
                    )
                    nc.vector.tensor_copy(
                        out=ob[:, e2 * 512 : (e2 + 1) * 512], in_=ps
                    )
                nc.sync.dma_start(out=out[tok0 : tok0 + P, :], in_=ob)

        # ---- program order ----
        # Software-pipelined: unit U's finish + output projection are emitted
        # interleaved into unit U+1's score/exp chunks, so ScalarE's exp
        # stream never drains at unit boundaries.  QKV chunks and V staging
        # are spread as filler between chunk groups.
        #
        # Unit (0,0) starts after only one QKV chunk (its first 4 score
        # chunks need K/V for tokens 0..511 only).
        qkv_chunk(0)
        v_chunk(0)
        ps00 = alloc_ps_a(0, 0)
        attention_chunks(0, 0, ps00, 0, 2)
        qkv_chunk(1)
        v_chunk(1)
        attention_chunks(0, 0, ps00, 2, 6)
        qkv_chunk(2)
        v_chunk(2)
        attention_chunks(0, 0, ps00, 6, 11)
        qkv_chunk(3)
        v_chunk(3)
        attention_chunks(0, 0, ps00, 11, 16)
        drain_unit(0, 0, ps00)

        # remaining filler work, scheduled into later units' chunk streams:
        #  unit (0,1): qkv 4, 5  + finish/outproj (0,0)
        #  unit (0,2): qkv 6, 7  + vtrans(1, 0..8)  + finish/outproj (0,1)
        #  unit (0,3): vtrans(1, 8..16) + finish/outproj (0,2)
        #  ...
        units = [(b, qq) for b in range(B) for qq in range(QQ)][1:]
        fill_tasks = {
            (0, 1): [lambda: qkv_chunk(4), lambda: v_chunk(4),
                     lambda: qkv_chunk(5), lambda: v_chunk(5)],
            (0, 2): [lambda: qkv_chunk(6), lambda: v_chunk(6),
                     lambda: qkv_chunk(7)],
            (0, 3): [lambda: v_chunk(7)],
        }
        prev = (0, 0)
        for (b, qq) in units:
            ps_a = alloc_ps_a(b, qq)
            tasks = list(fill_tasks.get((b, qq), []))
            pu = prev
            done = {"fin": False, "out": 0}

            def filler(j, tasks=tasks, pu=pu, done=done):
                # j in 0..15 over the unit's chunks
                if j == 1 and not done["fin"]:
                    finish_unit(*pu)
                    done["fin"] = True
                elif j >= 3 and j % 3 == 0 and done["out"] < 4:
                    outproj_chunk(pu, done["out"])
                    done["out"] += 1
                elif tasks:
                    tasks.pop(0)()

            def outproj_chunk(u, tci):
                ub, uqq = u
                qbase = ub * N + uqq * 512
                tok0 = qbase + tci * P
                ob = outst.tile([P, EMB], bf16, tag="outst")
                for e2 in range(2):
                    ps = ps_small.tile([P, 512], f32, tag="ps_small")
                    nc.tensor.matmul(
                        ps,
                        lhsT=anorm[:, tok0 : tok0 + P],
                        rhs=wout_sb[:, e2 * 512 : (e2 + 1) * 512],
                        start=True,
                        stop=True,
                    )
                    nc.vector.tensor_copy(
                        out=ob[:, e2 * 512 : (e2 + 1) * 512], in_=ps
                    )
                nc.sync.dma_start(out=out[tok0 : tok0 + P, :], in_=ob)

            attention_chunks(b, qq, ps_a, 0, KCH, filler=filler)
            # anything the filler didn't get to
            if not done["fin"]:
                finish_unit(*pu)
            while done["out"] < 4:
                outproj_chunk(pu, done["out"])
                done["out"] += 1
            for t in tasks:
                t()
            drain_unit(b, qq, ps_a)
            prev = (b, qq)

        # tail: last unit
        finish_unit(*prev)
        outproj_unit(*prev)


def _get_graph():
    if "nc" not in _CACHE:
        _CACHE["nc"] = _build_graph()
    return _CACHE["nc"]


def kernel(**inputs):
    x = np.asarray(inputs["x"], dtype=np.float32)
    W_qkv = np.asarray(inputs["W_qkv"], dtype=np.float32)
    b_qkv = np.asarray(inputs["b_qkv"], dtype=np.float32)
    W_out = np.asarray(inputs["W_out"], dtype=np.float32)
    b_out = np.asarray(inputs["b_out"], dtype=np.float32)

    nc = _get_graph()

    bf16 = ml_dtypes.bfloat16
    xT = np.ascontiguousarray(x.reshape(TOK, EMB).T).astype(bf16)
    in_maps = []
    for c in range(8):
        cols = np.concatenate(
            [
                np.arange(c * 128, (c + 1) * 128),
                np.arange(1024 + c * 128, 1024 + (c + 1) * 128),
                np.arange(2048 + c * 128, 2048 + (c + 1) * 128),
            ]
        )
        in_maps.append(
            {
                "xT": xT,
                "wqkv": np.ascontiguousarray(W_qkv[:, cols]).astype(bf16),
                "bqkv": np.ascontiguousarray(b_qkv[cols]).astype(np.float32),
                "wout": np.ascontiguousarray(
                    W_out[c * 128 : (c + 1) * 128, :]
                ).astype(bf16),
            }
        )

    from concourse.bass_utils import run_bass_kernel_spmd

    res = run_bass_kernel_spmd(nc, in_maps, core_ids=list(range(8)))
    LAST["results"] = res

    acc = np.zeros((TOK, EMB), np.float32)
    for r in res.results:
        acc += np.asarray(r["out"], dtype=np.float32)
    acc += b_out[None, :]
    # V-bias passes through softmax normalization as a constant add to the
    # attention output: attn @ (V + 1 b_v^T) / den = attn@V/den + b_v, so its
    # contribution to the output is just b_v @ W_out (the device kernel only
    # applies the K/Q biases).
    acc += b_qkv[2048:].astype(np.float32) @ W_out.astype(np.float32)
    return acc.reshape(B, N, EMB).astype(np.float32)


if __name__ == "__main__":
    rng = np.random.default_rng(0)
    inputs = {
        "x": rng.standard_normal((B, N, EMB), dtype=np.float32),
        "W_qkv": rng.standard_normal((EMB, 3072), dtype=np.float32) * EMB**-0.5,
        "b_qkv": np.zeros((3072,), np.float32),
        "W_out": rng.standard_normal((1024, EMB), dtype=np.float32) * 1024**-0.5,
        "b_out": np.zeros((1024,), np.float32),
    }
    y = kernel(**inputs)
    print("out", y.shape, y.dtype, float(np.abs(y).mean()))



# revision 9
# speedup vs baseline: 1.0469x; 1.0469x over previous
"""Multi-head attention (B=2, N=2048, EMB=1024, H=16, hd=64) on 8 TRN2 NeuronCores.

Sharding: tensor-parallel over heads. Each core owns 2 heads: it gets the
W_qkv columns (k|q|v sections) and W_out rows for those heads, computes
QKV projection + attention + its partial output projection, and the host
sums the 8 partials (the "all-reduce") and adds b_out.

Device kernel layout (per core), all matmuls bf16 with fp32 PSUM accumulation:
  - x is pre-transposed on host to xT [EMB, TOK] so the embedding dim lands on
    SBUF partitions (matmul contraction dim).
  - QKV^T is produced in [dims, tokens] layout: lhsT = W chunk, rhs = xT chunk.
    K/Q sections stay transposed ([hd, tok]) for the scores matmul; the V
    section is produced directly in [tok, hd] tiles (vaug) via plain matmuls
    with lhsT = xT chunk.
  - vaug per (b, i, h): head 0 = [v dims @ cols 0..63 | ones @ col 64],
    head 1 = [ones @ col 63 | v dims @ cols 64..127].  attn@V therefore puts
    head-1 dims straight onto PSUM partitions 64..127 (and its softmax
    denominator on row 63), so the normalized output lands on the partitions
    the out-projection needs without any partition-shift DMA.
  - scores^T chunks [k_tok 128, q 512] per head via row-packed (K=64) matmuls
    (the two heads run concurrently in disjoint PE row groups), exp on ScalarE
    straight out of PSUM (scale=1/8 folded in, no max subtraction needed:
    scores ~ N(0,1)), bf16 expT.
  - attn@V: lhsT = vaug [k_tok 128, 128], accumulated over 16 k chunks, with
    a global 2-chunk lag behind the exp stream.  The chunk stream is a single
    software pipeline across ALL 8 (batch, q-quarter) units, so the PE never
    drains at unit boundaries.
  - normalize: psa evicted bf16 (h0 rows 0..64, h1 rows 63..127); den rows
    DMA-packed to one [2, 512] tile; one Ln + one Exp (scale=-1) per unit on
    ScalarE (same table set as the bulk exp); partition-broadcast by a single
    K=2 bf16 selector-matmul; multiplies on VectorE into A_norm bf16.
  - out projection: lhsT = A_norm chunk [128, 128], rhs = W_out shard
    [128, 512], PSUM -> SBUF bf16 -> DRAM partial [TOK, EMB] bf16.
"""

import os
from collections import deque

import numpy as np
import ml_dtypes

B = 2
N = 2048
EMB = 1024
TOK = B * N  # 4096
HD = 64
H_PER_CORE = 2
DIMS = 3 * H_PER_CORE * HD  # 384 qkv cols per core
ATT_LOCAL = H_PER_CORE * HD  # 128
P = 128
EC = EMB // P  # 8 embedding chunks
TCQ = TOK // 512  # 8 token chunks for the qkv projection
KCH = N // P  # 16 key chunks per batch
QQ = N // 512  # 4 query quarters per batch
NUNITS = B * QQ  # 8
GCH = NUNITS * KCH  # 128 global chunks
SCALE = HD ** -0.5

_CACHE = {}
LAST = {}


def _patch_act_tables():
    # Route Exp to natural_log_exp_and_others so the per-unit Ln calls and
    # the bulk Exp calls share one table set (no ACT_TABLE_LOAD thrash).
    # Entries keep their order, so act_func_set_id indices stay valid.
    import concourse.bacc as bacc_mod
    from concourse import mybir

    if getattr(bacc_mod, "_act_tables_patched", False):
        return
    orig = bacc_mod.get_activation_tables

    def patched(arch):
        t = orig(arch)
        E = mybir.ActivationFunctionType.Exp
        if "natural_log_exp_and_others" in t:
            for name, fns in t.items():
                if name != "natural_log_exp_and_others" and E in fns:
                    t[name] = fns - {E}
        return t

    bacc_mod.get_activation_tables = patched
    bacc_mod._act_tables_patched = True


def _build_graph():
    from concourse import bacc, mybir
    import concourse.tile as tile

    _patch_act_tables()

    nc = bacc.Bacc(
        "TRN2", target_bir_lowering=False, debug=False, num_devices=1
    )
    dt = mybir.dt
    xT = nc.dram_tensor("xT", [EMB, TOK], dt.bfloat16, kind="ExternalInput")
    wqkv = nc.dram_tensor("wqkv", [EMB, DIMS], dt.bfloat16, kind="ExternalInput")
    bqkv = nc.dram_tensor("bqkv", [DIMS], dt.float32, kind="ExternalInput")
    wout = nc.dram_tensor("wout", [ATT_LOCAL, EMB], dt.bfloat16, kind="ExternalInput")
    out = nc.dram_tensor("out", [TOK, EMB], dt.bfloat16, kind="ExternalOutput")

    with tile.TileContext(nc) as tc:
        _emit(tc, nc, xT, wqkv, bqkv, wout, out)
    nc.compile()
    return nc


def _emit(tc, nc, xT, wqkv, bqkv, wout, out):
    from contextlib import ExitStack
    import concourse.bass as bass
    from concourse import mybir
    from concourse.masks import make_identity

    dt = mybir.dt
    f32, bf16 = dt.float32, dt.bfloat16
    Exp = mybir.ActivationFunctionType.Exp
    Ln = mybir.ActivationFunctionType.Ln

    with ExitStack() as ctx:
        consts = ctx.enter_context(tc.tile_pool(name="consts", bufs=1))
        xt_pool = ctx.enter_context(tc.tile_pool(name="xt", bufs=3))
        persist = ctx.enter_context(tc.tile_pool(name="persist", bufs=1))
        expp = ctx.enter_context(tc.tile_pool(name="expp", bufs=6))
        small = ctx.enter_context(tc.tile_pool(name="small", bufs=8))
        outst = ctx.enter_context(tc.tile_pool(name="outst", bufs=4))
        ps_scores = ctx.enter_context(
            tc.tile_pool(name="ps_scores", bufs=2, space="PSUM")
        )
        ps_att = ctx.enter_context(tc.tile_pool(name="ps_att", bufs=2, space="PSUM"))
        ps_small = ctx.enter_context(
            tc.tile_pool(name="ps_small", bufs=2, space="PSUM")
        )

        # ---- early DMAs: first xT chunk + weights, so the QKV stream can
        # start the moment the warmup matmuls finish ----
        xts = {}

        def dma_xt(t):
            xt = xt_pool.tile([P, EC, 512], bf16, tag="xt")
            for e in range(EC):
                nc.sync.dma_start(
                    out=xt[:, e, :], in_=xT[e * P : (e + 1) * P, bass.ts(t, 512)]
                )
            xts[t] = xt

        dma_xt(0)
        w_sb = consts.tile([P, EC, DIMS], bf16, tag="w_sb")
        for e in range(EC):
            nc.sync.dma_start(out=w_sb[:, e, :], in_=wqkv[e * P : (e + 1) * P, :])
        bias_sb = consts.tile([P, 3], f32, tag="bias_sb")
        nc.sync.dma_start(out=bias_sb, in_=bqkv[:].rearrange("(c p) -> p c", p=P))
        dma_xt(1)
        wout_sb = consts.tile([P, EMB], bf16, tag="wout_sb")
        nc.sync.dma_start(out=wout_sb, in_=wout[:, :])

        # warm up the exp table set as early as possible (one-time ~2.7us)
        warm = consts.tile([1, 8], f32, tag="warm")
        nc.vector.memset(warm, 1.0)
        nc.scalar.activation(out=warm, in_=warm, func=Ln, scale=1.0)
        nc.scalar.activation(out=warm, in_=warm, func=Exp, scale=1.0)

        ident = consts.tile([P, P], bf16, tag="ident")
        make_identity(nc, ident)
        # selector for the 1/den partition-broadcast: rows 0..63 get rc2[0]
        # (head 0), rows 64..127 get rc2[1] (head 1)
        # engine ops need 32-aligned partition bases, so row 1 of sel is
        # staged on partition 0 and DMA'd into place
        sel = consts.tile([2, P], bf16, tag="sel")
        nc.vector.memset(sel, 0.0)
        nc.vector.memset(sel[0:1, 0:HD], 1.0)
        selrow = consts.tile([1, P], bf16, tag="selrow")
        nc.vector.memset(selrow, 0.0)
        nc.vector.memset(selrow[0:1, HD:P], 1.0)
        nc.sync.dma_start(out=sel[1:2, :], in_=selrow)

        # [tok 128, b, chunk, h, 128]: h0 = [dims | ones@64 | junk],
        # h1 = [junk | ones@63 | dims@64..127].  Junk columns only feed
        # output rows that are never read.
        vaug = persist.tile([P, B, KCH, H_PER_CORE, P], bf16, tag="vaug")
        nc.vector.memset(vaug[:, :, :, :, :], 0.0)
        nc.vector.memset(vaug[:, :, :, 0, HD : HD + 1], 1.0)
        nc.vector.memset(vaug[:, :, :, 1, HD - 1 : HD], 1.0)

        k_sb = persist.tile([P, TOK], bf16, tag="k_sb")
        q_sb = persist.tile([P, TOK], bf16, tag="q_sb")
        anorm = persist.tile([P, TOK], bf16, tag="anorm")

        # ---- PE warmup: identity matmuls keep the PE busy through the HAM
        # activity window while the first DMAs land, so the QKV stream runs
        # at the warm 2.4 GHz clock from its first matmul ----
        wm_ps = ps_small.tile([P, 512], f32, tag="ps_small", name="wm_ps")
        for _ in range(44):
            nc.tensor.matmul(
                wm_ps[:, 0:128], lhsT=ident, rhs=ident, start=True, stop=True
            )

        qkv_dst = (k_sb, q_sb)

        def qkv_mms(t, d, e0, e1, ps):
            for e in range(e0, e1):
                nc.tensor.matmul(
                    ps,
                    lhsT=w_sb[:, e, d * P : (d + 1) * P],
                    rhs=xts[t][:, e, :],
                    start=(e == 0),
                    stop=(e == EC - 1),
                )

        def qkv_bias(t, d, ps):
            nc.vector.tensor_scalar_add(
                out=qkv_dst[d][:, bass.ts(t, 512)],
                in0=ps,
                scalar1=bias_sb[:, d : d + 1],
            )

        def v_mms(t, tci, ps):
            # V[tok, dim] for tokens t*512+tci*128 .. +128, accumulated into
            # column range tci*128..+128 of a shared [128, 512] PSUM bank
            # (one eviction copy per 4 sub-chunks).
            for e in range(EC):
                nc.tensor.matmul(
                    ps[:, tci * P : (tci + 1) * P],
                    lhsT=xts[t][:, e, tci * P : (tci + 1) * P],
                    rhs=w_sb[:, e, 2 * P : 3 * P],
                    start=(e == 0),
                    stop=(e == EC - 1),
                )

        def v_copy(t, ps):
            # ps cols [tci*128+h*64 .. +64] = head h dims for token sub-chunk
            # tci.  h0 -> vaug cols 0..64, h1 -> vaug cols 64..128.
            gtok = t * 512
            b, i0 = gtok // N, (gtok % N) // P
            src = ps.rearrange("p (c h d) -> p c h d", c=4, h=2)
            nc.vector.tensor_copy(
                out=vaug[:, b, i0 : i0 + 4, 0, 0:HD], in_=src[:, :, 0, :]
            )
            nc.vector.tensor_copy(
                out=vaug[:, b, i0 : i0 + 4, 1, HD:P], in_=src[:, :, 1, :]
            )

        # ---- filler task queue: (taskid, pe_cost_ns, fn) ----
        fillerq = deque()
        task_deadline = {}

        def enq_task(taskid, deadline, quanta):
            task_deadline[taskid] = deadline
            for cost, fn in quanta:
                fillerq.append((taskid, cost, fn))

        def run_quantum():
            _, _, fn = fillerq.popleft()
            fn()

        def pump(g, budget):
            # run everything whose deadline is upon us, then fill the budget
            while fillerq and task_deadline[fillerq[0][0]] <= g + 1:
                run_quantum()
            while fillerq and budget > 0:
                tid, cost, _ = fillerq[0]
                run_quantum()
                budget -= cost

        # Each quantum fully contains the lifetime of any ps_small tile it
        # allocates (alloc + all writes + final reader), so the pool's
        # round-robin reuse can never interleave with a half-written bank.
        def make_qkv_task(t):
            quanta = []
            for d in range(2):

                def q(d=d):
                    ps = ps_small.tile(
                        [P, 512], f32, tag="ps_small", name=f"qkv{t}_{d}"
                    )
                    qkv_mms(t, d, 0, EC, ps)
                    qkv_bias(t, d, ps)

                quanta.append((1760, q))
            return quanta

        def make_v_task(t):
            def q():
                ps = ps_small.tile([P, 512], f32, tag="ps_small", name=f"v{t}")
                for tci in range(4):
                    v_mms(t, tci, ps)
                v_copy(t, ps)

            return [(1800, q)]

        # t=0 runs inline before the stream; t=1..7 go through the queue.
        # Deadlines (global chunk index): batch-0 k-side needs qkv(t) by
        # chunk 4t; batch-1 needs qkv(4+j) by chunk 64+4j.  The xT DMA for
        # t+1 is enqueued ahead of qkv(t)'s matmuls so the transfer overlaps
        # a full task's worth of compute.
        def qkv_ddl(t):
            return 4 * t if t < 4 else 64 + 4 * (t - 4)

        for t in range(1, TCQ):
            if t + 1 < TCQ:
                enq_task(
                    f"dma{t + 1}",
                    max(qkv_ddl(t + 1) - 4, 0),
                    [(0, lambda t2=t + 1: dma_xt(t2))],
                )
            enq_task(f"qkv{t}", qkv_ddl(t), make_qkv_task(t))
            enq_task(f"v{t}", qkv_ddl(t) + 2, make_v_task(t))

        # ---- t=0 inline (the stream's first chunks need it) ----
        ps0 = ps_small.tile([P, 512], f32, tag="ps_small", name="qkv0_0")
        qkv_mms(0, 0, 0, EC, ps0)
        qkv_bias(0, 0, ps0)
        ps1 = ps_small.tile([P, 512], f32, tag="ps_small", name="qkv0_1")
        qkv_mms(0, 1, 0, EC, ps1)
        qkv_bias(0, 1, ps1)
        psv = ps_small.tile([P, 512], f32, tag="ps_small", name="v0")
        for tci in range(4):
            v_mms(0, tci, psv)
        v_copy(0, psv)

        # ---- per-unit finish machinery ----
        fin = {}

        def drain_unit(u, ps_a):
            # evict accumulators to SBUF (bf16): h0 rows 0..64 (den at 64),
            # h1 rows 63..127 (den at 63)
            psa0 = small.tile([P, 512], bf16, tag="psa_sb")
            nc.vector.tensor_copy(out=psa0[0 : HD + 1, :], in_=ps_a[0][0 : HD + 1, :])
            psa1 = small.tile([P, 512], bf16, tag="psa_sb")
            # non-zero partition bases are limited to 32-partition accesses,
            # and cost scales with the free dim only, so copy the full tile
            # (rows 0..62 are junk, den at 63, dims at 64..127)
            nc.vector.tensor_copy(out=psa1, in_=ps_a[1][:, :])
            den2 = small.tile([2, 512], bf16, tag="den2")
            nc.sync.dma_start(out=den2[0:1, :], in_=psa0[HD : HD + 1, :])
            nc.sync.dma_start(out=den2[1:2, :], in_=psa1[HD - 1 : HD, :])
            fin[u] = (psa0, psa1, den2)

        def finish_unit(u):
            # 1/den = exp(-ln(den)) on ScalarE, one [2, 512] call per func;
            # partition-broadcast via a single K=2 selector matmul; then
            # normalize multiplies on VectorE.
            psa0, psa1, den2 = fin.pop(u)
            b, qq = divmod(u, QQ)
            qbase = b * N + qq * 512
            tln = small.tile([2, 512], f32, tag="tln")
            nc.scalar.activation(out=tln, in_=den2, func=Ln)
            rc2 = small.tile([2, 512], bf16, tag="rc2")
            nc.scalar.activation(out=rc2, in_=tln, func=Exp, scale=-1.0)
            rrep = ps_small.tile([P, 512], f32, tag="ps_small", name=f"rrep{u}")
            nc.tensor.matmul(rrep, lhsT=sel, rhs=rc2, start=True, stop=True)
            nc.vector.tensor_mul(
                out=anorm[0:HD, qbase : qbase + 512],
                in0=psa0[0:HD, :],
                in1=rrep[0:HD, :],
            )
            # >32-partition accesses must start at partition 0, so the h1
            # multiply goes in two 32-partition pieces
            for p0 in (HD, HD + 32):
                nc.vector.tensor_mul(
                    out=anorm[p0 : p0 + 32, qbase : qbase + 512],
                    in0=psa1[p0 : p0 + 32, :],
                    in1=rrep[p0 : p0 + 32, :],
                )

        def outproj_chunk(u, tci):
            b, qq = divmod(u, QQ)
            tok0 = b * N + qq * 512 + tci * P
            ob = outst.tile([P, EMB], bf16, tag="outst")
            for e2 in range(2):
                ps = ps_small.tile([P, 512], f32, tag="ps_small")
                nc.tensor.matmul(
                    ps,
                    lhsT=anorm[:, tok0 : tok0 + P],
                    rhs=wout_sb[:, e2 * 512 : (e2 + 1) * 512],
                    start=True,
                    stop=True,
                )
                nc.vector.tensor_copy(out=ob[:, e2 * 512 : (e2 + 1) * 512], in_=ps)
            nc.sync.dma_start(out=out[tok0 : tok0 + P, :], in_=ob)

        # ---- the global chunk stream ----
        # One software pipeline over all 128 (unit, k-chunk) pairs: attn@V
        # lags the exp stream by 2 chunks and flows straight across unit
        # boundaries, so the PE never waits for an exp at a boundary.
        ps_a = {}
        lag = []

        def attnv(u, i, ex):
            b = u // QQ
            for h in range(H_PER_CORE):
                nc.tensor.matmul(
                    ps_a[u][h],
                    lhsT=vaug[:, b, i, h, :],
                    rhs=ex[:, h * 512 : (h + 1) * 512],
                    start=(i == 0),
                    stop=(i == KCH - 1),
                )
            if i == KCH - 1:
                drain_unit(u, ps_a.pop(u))

        for g in range(GCH):
            u, i = divmod(g, KCH)
            b, qq = divmod(u, QQ)
            if i == 0:
                ps_a[u] = [
                    ps_att.tile([P, 512], f32, tag="ps_att", name=f"ps_a{u}_{h}")
                    for h in range(H_PER_CORE)
                ]
            kbase = b * N + i * P
            qbase = b * N + qq * 512
            ps_s = ps_scores.tile([P, 1024], f32, tag="ps_s")
            for h in range(H_PER_CORE):
                nc.tensor.matmul(
                    ps_s[:, h * 512 : (h + 1) * 512],
                    lhsT=k_sb[h * HD : (h + 1) * HD, kbase : kbase + P],
                    rhs=q_sb[h * HD : (h + 1) * HD, qbase : qbase + 512],
                    start=True,
                    stop=True,
                )
            ex = expp.tile([P, 1024], bf16, tag="expT")
            nc.scalar.activation(out=ex, in_=ps_s, func=Exp, scale=SCALE)
            lag.append((u, i, ex))

            # unit-boundary bookkeeping rides the stream as filler tasks
            if i == 3 and u >= 1:
                enq_task(f"fin{u - 1}", g + 1, [(220, lambda v=u - 1: finish_unit(v))])
                enq_task(
                    f"out{u - 1}",
                    g + KCH - 3,
                    [
                        (470, lambda v=u - 1, tci=tci: outproj_chunk(v, tci))
                        for tci in range(4)
                    ],
                )

            pump(g, 450)
            if len(lag) > 2:
                attnv(*lag.pop(0))

        # ---- tail ----
        for item in lag:
            attnv(*item)
        while fillerq:
            run_quantum()
        finish_unit(NUNITS - 1)
        for tci in range(4):
            outproj_chunk(NUNITS - 1, tci)


def _get_graph():
    if "nc" not in _CACHE:
        _CACHE["nc"] = _build_graph()
    return _CACHE["nc"]


def kernel(**inputs):
    x = np.asarray(inputs["x"], dtype=np.float32)
    W_qkv = np.asarray(inputs["W_qkv"], dtype=np.float32)
    b_qkv = np.asarray(inputs["b_qkv"], dtype=np.float32)
    W_out = np.asarray(inputs["W_out"], dtype=np.float32)
    b_out = np.asarray(inputs["b_out"], dtype=np.float32)

    nc = _get_graph()

    bf16 = ml_dtypes.bfloat16
    xT = np.ascontiguousarray(x.reshape(TOK, EMB).T).astype(bf16)
    in_maps = []
    for c in range(8):
        cols = np.concatenate(
            [
                np.arange(c * 128, (c + 1) * 128),
                np.arange(1024 + c * 128, 1024 + (c + 1) * 128),
                np.arange(2048 + c * 128, 2048 + (c + 1) * 128),
            ]
        )
        in_maps.append(
            {
                "xT": xT,
                "wqkv": np.ascontiguousarray(W_qkv[:, cols]).astype(bf16),
                "bqkv": np.ascontiguousarray(b_qkv[cols]).astype(np.float32),
                "wout": np.ascontiguousarray(
                    W_out[c * 128 : (c + 1) * 128, :]
                ).astype(bf16),
            }
        )

    from concourse.bass_utils import run_bass_kernel_spmd

    res = run_bass_kernel_spmd(nc, in_maps, core_ids=list(range(8)))
    LAST["results"] = res

    acc = np.zeros((TOK, EMB), np.float32)
    for r in res.results:
        acc += np.asarray(r["out"], dtype=np.float32)
    acc += b_out[None, :]
    # V-bias passes through softmax normalization as a constant add to the
    # attention output: attn @ (V + 1 b_v^T) / den = attn@V/den + b_v, so its
    # contribution to the output is just b_v @ W_out (the device kernel only
    # applies the K/Q biases).
    acc += b_qkv[2048:].astype(np.float32) @ W_out.astype(np.float32)
    return acc.reshape(B, N, EMB).astype(np.float32)


if __name__ == "__main__":
    rng = np.random.default_rng(0)
    inputs = {
        "x": rng.standard_normal((B, N, EMB), dtype=np.float32),
        "W_qkv": rng.standard_normal((EMB, 3072), dtype=np.float32) * EMB**-0.5,
        "b_qkv": np.zeros((3072,), np.float32),
        "W_out": rng.standard_normal((1024, EMB), dtype=np.float32) * 1024**-0.5,
        "b_out": np.zeros((1024,), np.float32),
    }
    y = kernel(**inputs)
    print("out", y.shape, y.dtype, float(np.abs(y).mean()))
